# revision 10
# baseline (speedup 1.0000x reference)
"""CoLA encoder layer on 8 Trainium2 NeuronCores, data-parallel over batch.

v3: logits computed transposed.  S^T = M^T @ x_chunk lands as [ALPHA, CH]
in PSUM via 8 N=512 matmuls per chunk (vs 256 N=64 matmuls per pass in v1,
which were LDWEIGHTS-bound at ~107ns each).  exp(S^T/sqrt(D)) is E^T, which
is directly the stationary operand for the attention matmuls - the A^T PE
transposes are gone.  Softmax normalization (and the mask) commute through
the attention matmul as a per-token scalar, applied during the residual add:
r = attn_raw / (rowsum/mask) + x.  Rowsums come from a ones-column matmul
sharing the E^T stationary with the attention matmuls.

Per core (one batch element, L=4096, D=1024, ALPHA=64):
  S^T = M^T@x (bf16), E^T = exp, attn_raw = E@C_V^T + rowsums (fp32 PSUM),
  residual+LN1, z^T via PE transpose, pointwise conv, LeakyReLU, LN2.
"""

import sys

for _p in ("/opt/trn_rl_repo",):
    if _p not in sys.path:
        sys.path.insert(0, _p)

from contextlib import ExitStack

import ml_dtypes
import numpy as np

import concourse.bass as bass
import concourse.tile as tile
from concourse import bacc, mybir
from concourse.bass_utils import run_bass_kernel_spmd

F32 = mybir.dt.float32
F32R = mybir.dt.float32r
BF16 = mybir.dt.bfloat16
AF = mybir.ActivationFunctionType
ALU = mybir.AluOpType
AX = mybir.AxisListType

B, L, D, ALPHA = 8, 4096, 1024, 64
EPS = 1e-6
INV_SQRT_D = 1.0 / 32.0
CH = 512           # tokens per chunk
NCH = L // CH      # 8 chunks
LTPC = CH // 128   # l-tiles per chunk
DT = D // 128      # feature tiles

_CACHE = {}


def _build(g1_trivial: bool, g2_trivial: bool, bq_trivial: bool = True,
           time_iters: int = 1):
    nc = bacc.Bacc("TRN2", target_bir_lowering=False, debug=False)

    x_d = nc.dram_tensor("x", [L, D], BF16, kind="ExternalInput")
    xt_d = nc.dram_tensor("xt", [D, L], BF16, kind="ExternalInput")
    m_d = nc.dram_tensor("m", [D, ALPHA], BF16, kind="ExternalInput")
    cv_d = nc.dram_tensor("cv", [ALPHA, D], BF16, kind="ExternalInput")
    wc_d = nc.dram_tensor("wc", [D, D], BF16, kind="ExternalInput")
    mask_d = nc.dram_tensor("maskf", [128, L // 128], F32, kind="ExternalInput")
    id_d = nc.dram_tensor("ident", [128, 128], BF16, kind="ExternalInput")
    bcr_d = nc.dram_tensor("bcr", [1, D], F32, kind="ExternalInput")
    ones_d = nc.dram_tensor("ones64", [ALPHA, 1], BF16, kind="ExternalInput")
    if not bq_trivial:
        cs_d = nc.dram_tensor("cs64", [ALPHA, 1], F32, kind="ExternalInput")
    if not g1_trivial:
        g1_d = nc.dram_tensor("g1r", [1, D], F32, kind="ExternalInput")
        be1_d = nc.dram_tensor("be1r", [1, D], F32, kind="ExternalInput")
    if not g2_trivial:
        g2_d = nc.dram_tensor("g2r", [1, D], F32, kind="ExternalInput")
        be2_d = nc.dram_tensor("be2r", [1, D], F32, kind="ExternalInput")
    out_d = nc.dram_tensor("out", [L, D], BF16, kind="ExternalOutput")

    x_ap = x_d.ap()
    xt_ap = xt_d.ap().rearrange("(t p) l -> p t l", p=128)
    out_ap = out_d.ap()

    with tile.TileContext(nc) as tc:
        with ExitStack() as ctx:
            wp = ctx.enter_context(tc.tile_pool(name="weights", bufs=1))
            xt_pool = ctx.enter_context(tc.tile_pool(name="xtp", bufs=2))
            ht_pool = ctx.enter_context(tc.tile_pool(name="htp", bufs=2))
            xz_pool = ctx.enter_context(tc.tile_pool(name="xzp", bufs=8))
            zb_pool = ctx.enter_context(tc.tile_pool(name="zbp", bufs=12))
            ya_pool = ctx.enter_context(tc.tile_pool(name="yap", bufs=5))
            a_pool = ctx.enter_context(tc.tile_pool(name="ap", bufs=2))
            st_pool = ctx.enter_context(tc.tile_pool(name="stp", bufs=28))
            sq_pool = ctx.enter_context(tc.tile_pool(name="sqp", bufs=1))
            cb_pool = ctx.enter_context(tc.tile_pool(name="cbp", bufs=2))
            yb_pool = ctx.enter_context(tc.tile_pool(name="ybp", bufs=5))
            # PSUM banks: S^T 1 + rowsums 1 + zT 2 + (attn|conv) 2x2 = 8
            ps_st = ctx.enter_context(tc.tile_pool(name="psst", bufs=1, space="PSUM"))
            ps_sm = ctx.enter_context(tc.tile_pool(name="pssm", bufs=1, space="PSUM"))
            ps_mm = ctx.enter_context(tc.tile_pool(name="psmm", bufs=2, space="PSUM"))
            ps_big = ctx.enter_context(tc.tile_pool(name="psbig", bufs=2, space="PSUM"))

            m_sb = wp.tile([128, DT, ALPHA], BF16)
            nc.sync.dma_start(m_sb, m_d.ap().rearrange("(t p) a -> p t a", p=128))
            cv_sb = wp.tile([ALPHA, D], BF16)
            nc.sync.dma_start(cv_sb, cv_d.ap())
            id_sb = wp.tile([128, 128], BF16)
            nc.sync.dma_start(id_sb, id_d.ap())
            bc_sb = wp.tile([128, D], F32)
            nc.sync.dma_start(bc_sb, bcr_d.ap().to_broadcast((128, D)))
            mask_sb = wp.tile([128, L // 128], F32)
            nc.sync.dma_start(mask_sb, mask_d.ap())
            ones_sb = wp.tile([ALPHA, 1], BF16)
            nc.sync.dma_start(ones_sb, ones_d.ap())
            if not bq_trivial:
                cs_sb = wp.tile([ALPHA, 1], F32)
                nc.sync.dma_start(cs_sb, cs_d.ap())
            if not g1_trivial:
                g1_sb = wp.tile([128, D], F32)
                nc.sync.dma_start(g1_sb, g1_d.ap().to_broadcast((128, D)))
                be1_sb = wp.tile([128, D], F32)
                nc.sync.dma_start(be1_sb, be1_d.ap().to_broadcast((128, D)))
            if not g2_trivial:
                g2_sb = wp.tile([128, D], F32)
                nc.sync.dma_start(g2_sb, g2_d.ap().to_broadcast((128, D)))
                be2_sb = wp.tile([128, D], F32)
                nc.sync.dma_start(be2_sb, be2_d.ap().to_broadcast((128, D)))
            # big conv weight last so it doesn't block the x/xt stream;
            # issue from the Act queue to overlap with SP-queue input DMA
            wc_sb = wp.tile([128, DT, D], BF16)
            nc.scalar.dma_start(wc_sb, wc_d.ap().rearrange("(t p) e -> p t e", p=128))

            def issue_dma(c):
                """Input DMAs for chunk c, issued one chunk ahead."""
                l0 = c * CH
                xt_sb = xt_pool.tile([128, DT, CH], BF16, tag="xt")
                nc.sync.dma_start(xt_sb, xt_ap[:, :, l0 : l0 + CH])
                xz = []
                for lt in range(LTPC):
                    t = xz_pool.tile([128, D], BF16, name=f"xz{lt}", tag="xz")
                    nc.sync.dma_start(
                        t, x_ap[l0 + lt * 128 : l0 + (lt + 1) * 128, :]
                    )
                    xz.append(t)
                return xt_sb, xz

            def st_block(xt_sb):
                """S^T = M^T@x into one PSUM bank, then E^T = exp on ACT.
                Inputs were DMA'd a chunk ago, so the PE never waits here."""
                pst = ps_st.tile([ALPHA, CH], F32, name="pst", tag="pst")
                for d in range(DT):
                    nc.tensor.matmul(
                        pst,
                        m_sb[:, d, :],
                        xt_sb[:, d, :],
                        start=(d == 0),
                        stop=(d == DT - 1),
                    )
                # E^T = exp(S^T/32); logits are tiny (|S|/32 < 0.25) so the
                # max-subtraction is unnecessary.  b_Q folds in as a
                # per-partition bias.
                et = a_pool.tile([ALPHA, CH], BF16, name="et", tag="et")
                if bq_trivial:
                    nc.scalar.activation(et, pst, AF.Exp, scale=INV_SQRT_D)
                else:
                    nc.scalar.activation(
                        et, pst, AF.Exp, bias=cs_sb, scale=INV_SQRT_D
                    )
                return et

            def frontend_b(c, xz, et):
                """rowsums, attn, residual, LN1 -> zb."""
                # rowsums first (ones-column matmuls), then the per-token
                # scale mask/rowsum batched, so the residual can fire the
                # moment the attention matmul lands (pa-bank release is not
                # gated on small DVE ops)
                psums = ps_sm.tile([128, LTPC], F32, name="psums", tag="psums")
                for lt in range(LTPC):
                    nc.tensor.matmul(
                        psums[:, lt : lt + 1],
                        et[:, lt * 128 : (lt + 1) * 128],
                        ones_sb,
                        start=True,
                        stop=True,
                    )
                rcp = st_pool.tile([128, LTPC], F32, name="rcp", tag="rcp")
                nc.vector.reciprocal(rcp, psums)
                smask = st_pool.tile([128, LTPC], F32, name="smask", tag="smask")
                nc.vector.tensor_tensor(
                    smask, rcp, mask_sb[:, c * LTPC : (c + 1) * LTPC], ALU.mult
                )
                sum1 = st_pool.tile([128, LTPC], F32, name="sum1", tag="sum1")
                ssq1 = st_pool.tile([128, LTPC], F32, name="ssq1", tag="ssq1")
                for lt in range(LTPC):
                    ett = et[:, lt * 128 : (lt + 1) * 128]
                    pa = ps_big.tile([128, D], F32, name="pa", tag="mm1024")
                    for hf in range(2):
                        nc.tensor.matmul(
                            pa[:, hf * 512 : (hf + 1) * 512],
                            ett,
                            cv_sb[:, hf * 512 : (hf + 1) * 512],
                            start=True,
                            stop=True,
                        )
                    # residual: r = pa*smask + x; accum_out gives sum(r)
                    nc.vector.scalar_tensor_tensor(
                        xz[lt], pa, smask[:, lt : lt + 1], xz[lt],
                        ALU.mult, ALU.add,
                        accum_out=sum1[:, lt : lt + 1],
                    )
                    sq = sq_pool.tile([128, D], F32, name="sq", tag="sq")
                    nc.scalar.activation(
                        sq, xz[lt], AF.Square, accum_out=ssq1[:, lt : lt + 1]
                    )

                # LN1 stats, batched [128, 4]: mean = sum/D,
                # var = (ssq - D*mean^2)/(D-1), inv = 1/(sqrt(var)+eps)
                mean1 = st_pool.tile([128, LTPC], F32, name="mean1", tag="mean1")
                nc.vector.tensor_scalar_mul(mean1, sum1, 1.0 / D)
                var1 = st_pool.tile([128, LTPC], F32, name="var1", tag="var1")
                nc.vector.tensor_mul(var1, mean1, mean1)
                nc.vector.scalar_tensor_tensor(
                    var1, var1, -float(D), ssq1, ALU.mult, ALU.add
                )
                sd1 = st_pool.tile([128, LTPC], F32, name="sd1", tag="sd1")
                nc.scalar.activation(sd1, var1, AF.Sqrt, scale=1.0 / (D - 1))
                nc.vector.tensor_scalar_add(sd1, sd1, EPS)
                iv1 = st_pool.tile([128, LTPC], F32, name="iv1", tag="iv1")
                nc.vector.reciprocal(iv1, sd1)
                zb = []
                for lt in range(LTPC):
                    z = zb_pool.tile([128, D], BF16, name=f"zb{lt}", tag="zb")
                    nc.vector.tensor_scalar(
                        z, xz[lt], mean1[:, lt : lt + 1],
                        iv1[:, lt : lt + 1], ALU.subtract, ALU.mult,
                    )
                    if not g1_trivial:
                        nc.vector.tensor_mul(z, z, g1_sb)
                        nc.vector.tensor_add(z, z, be1_sb)
                    zb.append(z)
                return zb

            def backend_pre(c, zb):
                """z^T via PE transpose (bf16, 1 cyc/row), PSUM->SBUF copies."""
                ht_sb = ht_pool.tile([128, DT, CH], BF16, tag="ht")
                for j in range(DT // 2):
                    pzt = ps_mm.tile([128, 2, CH], BF16, name="pzt", tag="mm512")
                    for dj in range(2):
                        d = 2 * j + dj
                        for lt in range(LTPC):
                            nc.tensor.transpose(
                                pzt[:, dj, lt * 128 : (lt + 1) * 128],
                                zb[lt][:, d * 128 : (d + 1) * 128],
                                id_sb,
                            )
                    if j % 2 == 0:
                        nc.scalar.activation(
                            ht_sb[:, 2 * j : 2 * j + 2, :], pzt, AF.Copy
                        )
                    else:
                        nc.vector.tensor_copy(
                            ht_sb[:, 2 * j : 2 * j + 2, :], pzt
                        )
                return ht_sb

            def backend_post(c, zb, ht_sb):
                """conv, bias, LeakyReLU, residual, LN2, DMA out."""
                l0 = c * CH
                sum2 = st_pool.tile([128, LTPC], F32, name="sum2", tag="sum2")
                ssq2 = st_pool.tile([128, LTPC], F32, name="ssq2", tag="ssq2")
                yas = []
                for lt in range(LTPC):
                    l1 = lt * 128
                    pc = ps_big.tile([128, D], F32, name="pc", tag="mm1024")
                    for hf in range(2):
                        pch = pc[:, hf * 512 : (hf + 1) * 512]
                        for d in range(DT):
                            nc.tensor.matmul(
                                pch,
                                ht_sb[:, d, l1 : l1 + 128],
                                wc_sb[:, d, hf * 512 : (hf + 1) * 512],
                                start=(d == 0),
                                stop=(d == DT - 1),
                            )
                    # bias add on DVE, then leaky on Act
                    cb = cb_pool.tile([128, D], F32, name="cb", tag="cb")
                    nc.vector.tensor_add(cb, pc, bc_sb)
                    nc.scalar.activation(cb, cb, AF.Lrelu, alpha=0.01)
                    ya = ya_pool.tile([128, D], F32, name="ya", tag="ya")
                    yas.append(ya)
                    nc.vector.scalar_tensor_tensor(
                        ya, cb, 0.0, zb[lt], ALU.add, ALU.add,
                        accum_out=sum2[:, lt : lt + 1],
                    )
                    sq2 = sq_pool.tile([128, D], F32, name="sq2", tag="sq")
                    nc.scalar.activation(
                        sq2, ya, AF.Square, accum_out=ssq2[:, lt : lt + 1]
                    )

                mean2 = st_pool.tile([128, LTPC], F32, name="mean2", tag="mean2")
                nc.vector.tensor_scalar_mul(mean2, sum2, 1.0 / D)
                var2 = st_pool.tile([128, LTPC], F32, name="var2", tag="var2")
                nc.vector.tensor_mul(var2, mean2, mean2)
                nc.vector.scalar_tensor_tensor(
                    var2, var2, -float(D), ssq2, ALU.mult, ALU.add
                )
                sd2 = st_pool.tile([128, LTPC], F32, name="sd2", tag="sd2")
                nc.scalar.activation(sd2, var2, AF.Sqrt, scale=1.0 / (D - 1))
                nc.vector.tensor_scalar_add(sd2, sd2, EPS)
                iv2 = st_pool.tile([128, LTPC], F32, name="iv2", tag="iv2")
                nc.vector.reciprocal(iv2, sd2)
                for lt in range(LTPC):
                    ya = yas[lt]
                    yb = yb_pool.tile([128, D], BF16, name="yb", tag="yb")
                    nc.vector.tensor_scalar(
                        yb, ya, mean2[:, lt : lt + 1], iv2[:, lt : lt + 1],
                        ALU.subtract, ALU.mult,
                    )
                    if not g2_trivial:
                        nc.vector.tensor_mul(yb, yb, g2_sb)
                        nc.vector.tensor_add(yb, yb, be2_sb)
                    ya = yb
                    if time_iters > 1:
                        # accumulate so unrolled timing passes stay live
                        # (defeats dead-code elimination); SWDGE required
                        nc.gpsimd.dma_start(
                            out_ap[l0 + lt * 128 : l0 + (lt + 1) * 128, :],
                            ya, accum_op=ALU.add,
                        )
                    else:
                        nc.sync.dma_start(
                            out_ap[l0 + lt * 128 : l0 + (lt + 1) * 128, :], ya
                        )

            # 3-deep software pipeline: per iteration emit
            #   backend_pre(c-1): z^T transposes + PSUM copies
            #   frontend(c):      logits/softmax/attn/LN1
            #   backend_post(c-2): conv (reads ht copied a full iteration
            #                      earlier, so it never waits on copies)
            # time_iters>1 re-emits the whole body (straight-line unroll)
            # for steady-state timing.
            # Software pipeline over k = it*NCH + c.  Per k, PE-queue order:
            #   st(k)        S^T matmuls - inputs prefetched, never waits
            #   (exp(k) on ACT, issued immediately)
            #   zt(k-1)      z^T transposes - zb(k-1) ready since last chunk
            #   conv(k-2)    the big GEMM - covers exp(k)'s ACT latency
            #   attn(k)      rowsums+attn+residual+LN1 - et(k) ready by now
            # so every PE instruction has its inputs produced ~a chunk ahead.
            K = time_iters * NCH
            st = {}
            st[0] = issue_dma(0)
            for k in range(K):
                c = k % NCH
                if k + 1 < K:
                    st[k + 1] = issue_dma((k + 1) % NCH)
                et_k = st_block(st[k][0])
                if k >= 1:
                    st[k - 1] += (backend_pre((k - 1) % NCH, st[k - 1][2]),)
                if k >= 2:
                    xt_o, xz_o, zb_o, ht_o = st.pop(k - 2)
                    backend_post((k - 2) % NCH, zb_o, ht_o)
                zb_k = frontend_b(c, st[k][1], et_k)
                st[k] = (st[k][0], st[k][1], zb_k)
            for k in (K - 2, K - 1):
                if k < 0 or k not in st:
                    continue
                if len(st[k]) == 3:
                    st[k] += (backend_pre(k % NCH, st[k][2]),)
                xt_o, xz_o, zb_o, ht_o = st.pop(k)
                backend_post(k % NCH, zb_o, ht_o)


    nc.compile()
    return nc


def _get_nc(g1_trivial, g2_trivial, bq_trivial):
    key = (g1_trivial, g2_trivial, bq_trivial)
    if key not in _CACHE:
        _CACHE[key] = _build(*key)
    return _CACHE[key]


def build_in_maps(x, mask, W_Q, b_Q, C_K, C_V, g1, be1, Wc, bc, g2, be2):
    """Host-side prep shared by kernel() and test harnesses.

    Returns (nc, in_maps)."""
    x = np.asarray(x, dtype=np.float32)
    mask = np.asarray(mask)
    W_Q = np.asarray(W_Q, dtype=np.float32)
    b_Q = np.asarray(b_Q, dtype=np.float32)
    C_K = np.asarray(C_K, dtype=np.float32)
    C_V = np.asarray(C_V, dtype=np.float32)
    g1 = np.asarray(g1, dtype=np.float32)
    be1 = np.asarray(be1, dtype=np.float32)
    Wc = np.asarray(Wc, dtype=np.float32)
    bc = np.asarray(bc, dtype=np.float32)
    g2 = np.asarray(g2, dtype=np.float32)
    be2 = np.asarray(be2, dtype=np.float32)

    g1_trivial = bool(np.all(g1 == 1.0) and np.all(be1 == 0.0))
    g2_trivial = bool(np.all(g2 == 1.0) and np.all(be2 == 0.0))
    bq_trivial = bool(np.all(b_Q == 0.0))
    nc = _get_nc(g1_trivial, g2_trivial, bq_trivial)

    # Q only feeds the logits, so W_Q/C_K collapse on the host
    m16 = np.ascontiguousarray(W_Q.T @ C_K).astype(ml_dtypes.bfloat16)
    wcT = np.ascontiguousarray(Wc.T).astype(ml_dtypes.bfloat16)
    cvT = np.ascontiguousarray(C_V.T).astype(ml_dtypes.bfloat16)
    bc_row = bc.reshape(1, D)
    ident = np.eye(128, dtype=np.float32)

    in_maps = []
    for b in range(B):
        m = {
            "x": np.ascontiguousarray(x[b].astype(ml_dtypes.bfloat16)),
            "xt": np.ascontiguousarray(x[b].T.astype(ml_dtypes.bfloat16)),
            "m": m16,
            "wc": wcT,
            "cv": cvT,
            "bcr": bc_row,
            "maskf": np.ascontiguousarray(
                mask[b].astype(np.float32).reshape(L // 128, 128).T
            ),
            "ident": ident.astype(ml_dtypes.bfloat16),
            "ones64": np.ones((ALPHA, 1), dtype=ml_dtypes.bfloat16),
        }
        if not bq_trivial:
            m["cs64"] = (
                (b_Q @ C_K) * INV_SQRT_D
            ).reshape(ALPHA, 1).astype(np.float32)
        if not g1_trivial:
            m["g1r"] = g1.reshape(1, D)
            m["be1r"] = be1.reshape(1, D)
        if not g2_trivial:
            m["g2r"] = g2.reshape(1, D)
            m["be2r"] = be2.reshape(1, D)
        in_maps.append(m)
    return nc, in_maps


def kernel(x, mask, W_Q, b_Q, C_K, C_V, g1, be1, Wc, bc, g2, be2):
    nc, in_maps = build_in_maps(
        x, mask, W_Q, b_Q, C_K, C_V, g1, be1, Wc, bc, g2, be2
    )
    res = run_bass_kernel_spmd(nc, in_maps, core_ids=list(range(B)))
    return np.stack(
        [np.asarray(res.results[b]["out"]) for b in range(B)], axis=0
    ).astype(np.float32)
